# revision 10
# baseline (speedup 1.0000x reference)
"""MFA per-component log-likelihood kernel for 8x TRN2 NeuronCores.

Math: reference computes, for K=128 mixture components with Woodbury
factor structure Sigma_k = D_k^2 + A_k A_k^T (via the small l=16 matrix
L = I + A^T iD A):

  out[n,k] = PI[k] - 0.5*(d*log2pi + logdetSigma[k] + m_d[n,k])
  m_d = term1 - quad,  term1 = sum_d iD (x-MU)^2,  quad = y^T iL y

Host-side (tiny, O(K*d*l)): compute iL = C C^T (Cholesky), fold
everything into three weight matrices so the device does only:

  out[n,k] = base[k] + sum_m (x_n @ Gw[:,k*16+m])^2 + x_n @ Wx[:,k]
             + (x_n^2) @ Wxx[:,k]

where (with G'_k = iD A C / sqrt(2), h'_k = (MU^T iD A C)/sqrt(2)):
  Gw[:, k*16+m] = G'_k[:, m]
  Wx[:, k]      = (iD*MU)_k - 2 * G'_k @ h'_k
  Wxx[:, k]     = -0.5 * iD_k
  base[k]       = PI[k] - 0.5*(d*log2pi + logdetSigma_k + sum_d iD MU^2)
                  + sum_m h'^2

Device: per 128-row tile of x (x pre-transposed on host so the
contraction dim d sits on SBUF partitions), 8 fp32 PE matmuls per
512-col block accumulate in PSUM; ScalarE squares PSUM->SBUF; VectorE
does the group-of-16 reduce and the final adds.  Sharding: rows N=16384
split across 8 cores (2048 rows each); params replicated.
"""

import math

import numpy as np

K, D_FEAT, L_FAC, N = 128, 1024, 16, 16384
N_CORES = 8
N_SHARD = N // N_CORES            # 2048 rows per core
NT = N_SHARD // 128               # 16 row tiles per core
DT = D_FEAT // 128                # 8 contraction tiles
GCOLS = K * L_FAC                 # 2048 factor columns
WCOLS = GCOLS + K                 # 2176 = [Gw | Wx]

_CACHE = {}


def _get_nc():
    if "nc" in _CACHE:
        return _CACHE["nc"]

    import concourse.bass as bass
    import concourse.tile as tile
    from concourse import bacc, mybir

    f32 = mybir.dt.float32
    f32r = mybir.dt.float32r
    nc = bacc.Bacc("TRN2", target_bir_lowering=False, debug=False,
                   num_devices=N_CORES)

    xT = nc.dram_tensor("xT", [D_FEAT, N_SHARD], f32, kind="ExternalInput").ap()
    wmov = nc.dram_tensor("wmov", [128, DT, WCOLS], f32, kind="ExternalInput").ap()
    wxx = nc.dram_tensor("wxx", [128, DT, K], f32, kind="ExternalInput").ap()
    baser = nc.dram_tensor("baser", [128, K], f32, kind="ExternalInput").ap()
    out = nc.dram_tensor("out", [N_SHARD, K], f32, kind="ExternalOutput").ap()

    with tile.TileContext(nc) as tc:
        with (
            tc.tile_pool(name="singles", bufs=1) as singles,
            tc.tile_pool(name="xtp", bufs=4) as xtp,
            tc.tile_pool(name="xsqp", bufs=4) as xsqp,
            tc.tile_pool(name="sqp", bufs=8) as sqp,
            tc.tile_pool(name="ep", bufs=3) as ep,
            tc.tile_pool(name="gps", bufs=4, space="PSUM") as gps,
            tc.tile_pool(name="tpsa", bufs=2, space="PSUM") as tpsa,
            tc.tile_pool(name="tpsb", bufs=2, space="PSUM") as tpsb,
        ):
            wm = singles.tile([128, DT, WCOLS], f32r, tag="wm")
            nc.sync.dma_start(out=wm, in_=wmov.bitcast(f32r))
            wx = singles.tile([128, DT, K], f32, tag="wx")
            nc.sync.dma_start(out=wx, in_=wxx)
            bs = singles.tile([128, K], f32, tag="bs")
            nc.sync.dma_start(out=bs, in_=baser)

            # The LDWEIGHTS instruction can carry only one semaphore wait,
            # so a matmul may depend on at most one not-yet-observed
            # processor.  Touch each weight tensor with a throwaway matmul
            # first so the real matmuls only ever wait on their xt DMA.
            warm = gps.tile([128, 512], f32, tag="g")
            nc.tensor.matmul(warm, wm[:, 0, 0:128], wm[:, 0, 0:512],
                             start=True, stop=True)
            warm2 = gps.tile([128, 512], f32, tag="g")
            nc.tensor.matmul(warm2[:, 0:128], wx[:, 0, 0:128], wx[:, 0, 0:128],
                             start=True, stop=True)
            # same trick for VectorE's first read of bs
            warm3 = ep.tile([128, 1], f32, tag="w3")
            nc.vector.tensor_copy(warm3, bs[:, 0:1])

            for i in range(NT):
                xt = xtp.tile([128, DT, 128], f32r, tag="xt")
                nc.sync.dma_start(
                    out=xt,
                    in_=xT[:, i * 128:(i + 1) * 128].rearrange(
                        "(j p) n -> p j n", p=128).bitcast(f32r),
                )
                xsq = xsqp.tile([128, DT, 128], f32, tag="xsq")
                nc.scalar.square(xsq, xt)

                quad = ep.tile([128, K], f32, tag="quad")
                for cb in range(4):
                    ps = gps.tile([128, 512], f32, tag="g")
                    for j in range(DT):
                        nc.tensor.matmul(
                            ps, xt[:, j, :],
                            wm[:, j, cb * 512:(cb + 1) * 512],
                            start=(j == 0), stop=(j == DT - 1))
                    sq = sqp.tile([128, 512], f32, tag="sq")
                    nc.scalar.square(sq, ps)
                    nc.vector.reduce_sum(
                        out=quad[:, cb * 32:(cb + 1) * 32],
                        in_=sq.rearrange("p (g i) -> p g i", i=L_FAC),
                        axis=mybir.AxisListType.X,
                    )

                psa = tpsa.tile([128, K], f32, tag="ta")
                for j in range(DT):
                    nc.tensor.matmul(psa, xt[:, j, :],
                                     wm[:, j, GCOLS:],
                                     start=(j == 0), stop=(j == DT - 1))
                psb = tpsb.tile([128, K], f32, tag="tb")
                for j in range(DT):
                    nc.tensor.matmul(psb, xsq[:, j, :],
                                     wx[:, j, :],
                                     start=(j == 0), stop=(j == DT - 1))

                u = ep.tile([128, K], f32, tag="u")
                nc.vector.tensor_add(out=u, in0=quad, in1=bs)
                nc.vector.tensor_add(out=u, in0=u, in1=psa)
                nc.vector.tensor_add(out=u, in0=u, in1=psb)
                nc.gpsimd.dma_start(out=out[i * 128:(i + 1) * 128, :], in_=u)

    nc.finalize()
    _CACHE["nc"] = nc
    return nc


def _host_params(PI, MU, A, D):
    PI64 = PI.astype(np.float64)
    MU64 = MU.astype(np.float64)
    A64 = A.astype(np.float64)
    D64 = D.astype(np.float64)

    iD = D64 ** -2.0                                   # (K, d)
    iDA = iD[:, :, None] * A64                         # (K, d, l)
    Lm = np.eye(L_FAC)[None] + np.einsum("kdl,kdm->klm", A64, iDA)
    iL = np.linalg.inv(Lm)
    C = np.linalg.cholesky(iL)                         # iL = C C^T
    s = 1.0 / math.sqrt(2.0)
    G = np.einsum("kdl,klm->kdm", iDA, C) * s          # (K, d, l)
    b = np.einsum("kd,kdl->kl", MU64, iDA)             # (K, l)
    h = np.einsum("kl,klm->km", b, C) * s              # (K, l)

    Gw = G.transpose(1, 0, 2).reshape(D_FEAT, GCOLS)   # col k*16+m
    Wx = (iD * MU64).T - 2.0 * np.einsum("kdm,km->kd", G, h).T
    Wxx = -0.5 * iD.T

    det_L = np.linalg.slogdet(Lm)[1]
    log_det_sigma = det_L - np.sum(np.log(iD), axis=1)
    c1 = np.sum(iD * MU64 * MU64, axis=1)
    hsq = np.sum(h * h, axis=1)
    base = PI64 - 0.5 * (D_FEAT * math.log(2.0 * math.pi)
                         + log_det_sigma + c1) + hsq

    wcat = np.concatenate([Gw, Wx], axis=1)            # (d, 2176)
    wmov = np.ascontiguousarray(
        wcat.reshape(DT, 128, WCOLS).transpose(1, 0, 2)).astype(np.float32)
    wxx = np.ascontiguousarray(
        Wxx.reshape(DT, 128, K).transpose(1, 0, 2)).astype(np.float32)
    baser = np.broadcast_to(base.astype(np.float32), (128, K)).copy()
    return wmov, wxx, baser


def kernel(x, PI, MU, A, D, _trace=False):
    from concourse.bass_utils import run_bass_kernel_spmd

    x = np.asarray(x, dtype=np.float32)
    wmov, wxx, baser = _host_params(
        np.asarray(PI), np.asarray(MU), np.asarray(A), np.asarray(D))

    in_maps = []
    for c in range(N_CORES):
        xs = x[c * N_SHARD:(c + 1) * N_SHARD]
        in_maps.append({
            "xT": np.ascontiguousarray(xs.T),
            "wmov": wmov,
            "wxx": wxx,
            "baser": baser,
        })

    nc = _get_nc()
    res = run_bass_kernel_spmd(nc, in_maps, list(range(N_CORES)),
                               trace=_trace)
    _CACHE["last_results"] = res
    return np.concatenate([res.results[c]["out"] for c in range(N_CORES)],
                          axis=0)
